# revision 33
# baseline (speedup 1.0000x reference)
"""Trainium2 Bass kernel for CodebookMapper (vq_codebook).

Full-input contract: kernel(x[32768,512] f32, codebook[8192,512] f32) ->
quantized[32768,512] f32, computing
    xn   = l2norm(x, axis=1)
    sims = xn @ codebook.T / 0.07
    soft = softmax(sims, axis=1)
    out  = soft @ codebook

Sharding: data-parallel over rows of x across 8 NeuronCores; codebook
replicated. Each core runs an identical NEFF on its 4096-row shard.

Per-core dataflow (bf16 matmuls, fp32 accumulation):
  setup: load codebook, cast to bf16 (cb_n, [k,d] layout), and build the
         transposed copy cb_t ([d,k] layout) with PE-mode transposes.
  per 128-row tile of x:
    1. normalize rows in fp32 (Square+accum on ACT, rsqrt via Sqrt+recip),
       cast to bf16
    2. PE-transpose xn -> xnT (lhsT for GEMM1)
    3. GEMM1: sims chunk [128,512] = xnT.T @ cb_t chunk, accumulated over
       d in PSUM; ACT applies exp(sims/tau) PSUM->SBUF bf16, with the
       per-row sum of each chunk accumulated for free via accum_out
    4. GEMM2: PE-transpose each exp tile [128,128] -> lhsT, accumulate
       q_unnorm [128,512] = sum_k expT.T @ cb_n[k] in a single PSUM bank
    5. softmax normalization folded into the epilogue:
       out = q_unnorm * (1/rowsum), DMA to DRAM

exp needs no max-subtraction: |logits| <= 1/tau = 14.3 so exp is in
[6e-7, 1.6e6], comfortably inside fp32/bf16 range.
"""

import numpy as np

import concourse.bass as bass
import concourse.tile as tile
from concourse import bacc, mybir
from concourse.bass_utils import run_bass_kernel_spmd
from concourse.masks import make_identity

N_CORES = 8
K_FULL = 8192
D_FULL = 512
TAU = 0.07

F32 = mybir.dt.float32
BF16 = mybir.dt.bfloat16
FP8 = mybir.dt.float8e4
AF = mybir.ActivationFunctionType
ALU = mybir.AluOpType


USE_DMA_TRANSPOSE = False
LAYOUT = "tr2"  # "tr": sims.T flow; "tr2": fp8 G2 w/ natural output layout
# scheduling knobs (numerics-identical)
SETUP_CAST_ENGINES = "g"  # "g": gpsimd; "va": alternate vector/scalar
PSUM_G1_BUFS = 2
PSUM_T_BUFS = 2
EXPT_BUFS = 6
PS_SHARE_PST = False  # alternate GEMM1T psum tiles into the pst pool
G2_DELAY = True       # emit GEMM2T(kk-1) after GEMM1T(kk) to hide exp latency
RACC_SPLIT = False    # two alternating racc accumulators + final combine
XNT_COPY_ENGINE = "v"  # "s": scalar/ACT; "v": vector/DVE (keeps ACT for exp)
PROLOGUE_SPLIT = 4    # number of emission points for next super's prologue
# fp8 GEMM2: exp weights + codebook quantized to fp8e4, DoubleRow matmuls
# (2 k-blocks per instruction, ~1.44x PE throughput). GEMM1 stays bf16 —
# fp8 logits fail the 2e-2 accuracy gate (error amplified by exp(x/0.07)).
G2_FP8 = True
CB8_SCALE = 32.0  # lifts codebook elems (~0.04) into fp8 normal range
# softmax denominators on the PE (tiny DoubleRow ones-matmul per pair into a
# dedicated PSUM bank) instead of a 64-op serial DVE accumulate chain; the
# row-norm square/reduce moves to the otherwise-idle gpsimd engine. Both
# keep ACT exp-only and DVE light so the PE is the sole bottleneck.
RS_ON_PE = False
NORM_ON_GPSIMD = True
G2_DELAY_PAIRS = 2   # tr2: pairs of slack between exp production and GEMM2
G2_INTERLEAVE = True  # tr2: interleave G2' matmuls between G1 dd-chains
XNT_DMA_TRANSPOSE = True  # tr2: xnT transposes on the (idle) DMA engine
# split the 64-chunk racc accumulation chain across DVE (even chunks) and
# gpsimd (odd chunks): ~38 us/super each instead of 77 us on DVE alone.
RACC_GPS_SPLIT = True
# timing-only ablations for bottleneck hunting: "" | "noact" | "nog2" | "nog1"
ABLATE = ""


def _build_kernel(tc: tile.TileContext, out_ap, x_ap, cb_aps, n_local, k, d,
                  reps=1):
    from contextlib import ExitStack

    if LAYOUT == "tr2" and n_local % 512 == 0:
        def inner(ctx):
            _build_kernel_inner_t2(ctx, tc, out_ap, x_ap, cb_aps[0],
                                   cb_aps[1], n_local, k, d)
    elif LAYOUT == "tr" and n_local % 512 == 0:
        def inner(ctx):
            _build_kernel_inner_t(ctx, tc, out_ap, x_ap, cb_aps[0],
                                  n_local, k, d)
    else:
        def inner(ctx):
            _build_kernel_inner(ctx, tc, out_ap, x_ap, cb_aps[0],
                                n_local, k, d)
    with ExitStack() as ctx:
        if reps > 1:
            # Timing harness: loop the whole kernel on-device so host /
            # axon dispatch overhead can be differenced away.
            with tc.For_i(0, reps, 1):
                inner(ctx)
        else:
            inner(ctx)


def _build_kernel_inner_t(ctx, tc, out_ap, x_ap, cb_ap, n_local, k, d):
    """Transposed-sims dataflow.

    GEMM1 produces simsT chunks [k128, m512] directly (lhsT = cb_t chunk,
    rhs = xnT), exp is applied in that layout, and GEMM2 consumes the exp
    chunk as the *moving* operand (lhsT = cb_n chunk), accumulating
    Q.T [d128, m512] across k in 4 PSUM banks. No per-chunk exp
    transposes. Softmax denominators: DVE accumulates sum_k expT chunks
    into racc [128, m], a ones-matmul folds the remaining 128 partitions,
    and tiny PE transposes turn [1, m] into per-partition [m, 1] for the
    output scale, applied while transposing Q.T back to natural layout.
    """
    nc = tc.nc
    P = 128
    KT = k // P          # 64
    DT = d // P          # 4
    MSUP = 512           # m super-tile = free dim of the transposed GEMMs
    MTS = MSUP // P      # 4
    MS = n_local // MSUP  # 8

    rs_on_pe = G2_FP8 and RS_ON_PE
    persist = ctx.enter_context(tc.tile_pool(name="persist", bufs=1))
    stage = ctx.enter_context(tc.tile_pool(name="stage", bufs=3))
    io_pool = ctx.enter_context(tc.tile_pool(name="io", bufs=2))
    expt_pool = ctx.enter_context(tc.tile_pool(name="expt", bufs=EXPT_BUFS))
    racc_pool = ctx.enter_context(tc.tile_pool(name="racc", bufs=2))
    small = ctx.enter_context(tc.tile_pool(name="small", bufs=4))
    psum_t = ctx.enter_context(
        tc.tile_pool(name="psum_t", bufs=1 if rs_on_pe else PSUM_T_BUFS,
                     space="PSUM"))
    psum_g1 = ctx.enter_context(
        tc.tile_pool(name="psum_g1", bufs=PSUM_G1_BUFS, space="PSUM"))
    psum_q = ctx.enter_context(tc.tile_pool(name="psum_q", bufs=1, space="PSUM"))
    psum_rs = (ctx.enter_context(tc.tile_pool(name="psum_rs", bufs=1,
                                              space="PSUM"))
               if rs_on_pe else None)

    ident = persist.tile([P, P], BF16)
    make_identity(nc, ident)
    ident_f = persist.tile([P, P], F32)
    make_identity(nc, ident_f)
    ones_f = persist.tile([P, 1], F32)
    # With fp8 GEMM2 the codebook carries a CB8_SCALE factor; fold its
    # inverse into the softmax denominator (rst = SCALE * rowsum).
    nc.vector.memset(ones_f, CB8_SCALE if G2_FP8 else 1.0)
    if rs_on_pe:
        # stationary for the rowsum DoubleRow matmul: [128, 2, 1] of
        # CB8_SCALE (exactly representable in fp8). 16-wide free dim keeps
        # the pair-dim AP step a multiple of 16 bytes.
        ones8 = persist.tile([P, 2, 16], FP8)
        nc.vector.memset(ones8, CB8_SCALE)

    cb_t = persist.tile([P, DT, k], BF16)
    if G2_FP8:
        # GEMM2 codebook in fp8 (scaled by CB8_SCALE; undone via ones_f in
        # the rowsum fold). bf16 copies only staged transiently for cb_t.
        cb8 = persist.tile([P, KT, d], FP8)
        cb_n = None
    else:
        cb8 = None
        cb_n = persist.tile([P, KT, d], BF16)
    for ko in range(KT):
        cst = stage.tile([P, d], F32)
        nc.sync.dma_start(cst, cb_ap[ko * P:(ko + 1) * P, :])
        if G2_FP8:
            cbb = stage.tile([P, d], BF16, name="cbb")
            nc.gpsimd.tensor_copy(cbb, cst)
            nc.vector.tensor_scalar_mul(cb8[:, ko, :], cst, CB8_SCALE)
        elif SETUP_CAST_ENGINES == "g":
            nc.gpsimd.tensor_copy(cb_n[:, ko, :], cst)
        else:
            eng = nc.vector if ko % 2 == 0 else nc.scalar
            if eng is nc.vector:
                nc.vector.tensor_copy(cb_n[:, ko, :], cst)
            else:
                nc.scalar.copy(cb_n[:, ko, :], cst)
        for dd in range(DT):
            src_n = (cbb[:, dd * P:(dd + 1) * P] if G2_FP8
                     else cb_n[:, ko, dd * P:(dd + 1) * P])
            tps = psum_t.tile([P, P], BF16, tag="pst")
            nc.tensor.transpose(tps, src_n, ident)
            nc.vector.tensor_copy(cb_t[:, dd, ko * P:(ko + 1) * P], tps)

    inv_tau = float(1.0 / TAU)

    def emit_norm_xnT_mt(s, mt, xnT):
        """Load + normalize m-tile mt of super-tile s into xnT [d, m]."""
        row0 = s * MSUP
        x_t = io_pool.tile([P, d], F32, name="x_t")
        nc.sync.dma_start(x_t, x_ap[row0 + mt * P:row0 + (mt + 1) * P, :])
        sq = io_pool.tile([P, d], F32, name="sq")
        ss = small.tile([P, 1], F32, name="ss")
        if NORM_ON_GPSIMD:
            nc.gpsimd.tensor_mul(sq, x_t, x_t)
            nc.vector.tensor_reduce(ss, sq, axis=mybir.AxisListType.X,
                                    op=ALU.add)
        else:
            nc.scalar.activation(out=sq, in_=x_t, func=AF.Square, accum_out=ss)
        nrm = small.tile([P, 1], F32, name="nrm")
        nc.scalar.sqrt(nrm, ss)
        rstd = small.tile([P, 1], F32, name="rstd")
        nc.vector.reciprocal(rstd, nrm)
        xn_b = io_pool.tile([P, d], BF16, name="xn_b")
        nc.vector.tensor_scalar_mul(xn_b, x_t, rstd)
        for dd in range(DT):
            xps = psum_t.tile([P, P], BF16, tag="pst", name="xps")
            nc.tensor.transpose(xps, xn_b[:, dd * P:(dd + 1) * P], ident)
            if XNT_COPY_ENGINE == "v":
                nc.vector.tensor_copy(xnT[:, dd, mt * P:(mt + 1) * P], xps)
            else:
                nc.scalar.copy(xnT[:, dd, mt * P:(mt + 1) * P], xps)

    def emit_norm_xnT(s):
        xnT = io_pool.tile([P, DT, MSUP], BF16, name="xnT")
        for mt in range(MTS):
            emit_norm_xnT_mt(s, mt, xnT)
        return xnT

    NPAIR = KT // 2

    def emit_g2(kk, et, qaccT):
        for dd in range(DT):
            nc.tensor.matmul(
                qaccT[:, dd, :],
                cb_n[:, kk, dd * P:(dd + 1) * P],
                et,
                start=(kk == 0),
                stop=(kk == KT - 1),
            )

    def emit_g2_pair(p, et8, qaccT, rs_ps):
        # DoubleRow: one instruction contracts 2 k-blocks; lhsT [128,2,128]
        # fp8 codebook pair, rhs [128,2,512] fp8 exp pair.
        for dd in range(DT):
            nc.tensor.matmul(
                qaccT[:, dd, :],
                cb8[:, 2 * p:2 * p + 2, dd * P:(dd + 1) * P],
                et8,
                start=(p == 0),
                stop=(p == NPAIR - 1),
                perf_mode=mybir.MatmulPerfMode.DoubleRow,
            )
        if rs_on_pe:
            # softmax denominators ride along: [1,512] += 32 * colsum(et8)
            nc.tensor.matmul(
                rs_ps[0:1, :],
                ones8[:, :, 0:1],
                et8,
                start=(p == 0),
                stop=(p == NPAIR - 1),
                perf_mode=mybir.MatmulPerfMode.DoubleRow,
            )

    def emit_kloop_segment(kk_range, xnT, qaccT, raccs, pending):
        for kk in kk_range:
            if PS_SHARE_PST and kk % 2 == 1:
                ps = psum_t.tile([P, MSUP], F32, tag="pst", name="ps")
            else:
                ps = psum_g1.tile([P, MSUP], F32, name="ps")
            for dd in range(DT):
                nc.tensor.matmul(
                    ps,
                    cb_t[:, dd, kk * P:(kk + 1) * P],
                    xnT[:, dd, :],
                    start=(dd == 0),
                    stop=(dd == DT - 1),
                )
            et = expt_pool.tile([P, MSUP], BF16, name="et")
            nc.scalar.activation(out=et, in_=ps, func=AF.Exp, scale=inv_tau)
            racc = raccs[kk % len(raccs)]
            if kk < len(raccs):
                nc.vector.tensor_copy(racc, et)
            else:
                nc.vector.tensor_add(racc, racc, et)
            if G2_DELAY:
                if pending is not None:
                    emit_g2(pending[0], pending[1], qaccT)
                pending = (kk, et)
            else:
                emit_g2(kk, et, qaccT)
        return pending

    def emit_kloop_segment_fp8(pair_range, xnT, qaccT, raccs, rs_ps, pending):
        for p in pair_range:
            et8 = expt_pool.tile([P, 2, MSUP], FP8, name="et8")
            for j in (0, 1):
                kk = 2 * p + j
                ps = psum_g1.tile([P, MSUP], F32, name="ps")
                for dd in range(DT):
                    nc.tensor.matmul(
                        ps,
                        cb_t[:, dd, kk * P:(kk + 1) * P],
                        xnT[:, dd, :],
                        start=(dd == 0),
                        stop=(dd == DT - 1),
                    )
                nc.scalar.activation(
                    out=et8[:, j, :], in_=ps, func=AF.Exp, scale=inv_tau)
                if not rs_on_pe:
                    racc = raccs[kk % len(raccs)]
                    eng = (nc.gpsimd if RACC_GPS_SPLIT and kk % 2 == 1
                           else nc.vector)
                    if kk < len(raccs):
                        eng.tensor_copy(racc, et8[:, j, :])
                    else:
                        eng.tensor_add(racc, racc, et8[:, j, :])
            if pending is not None:
                emit_g2_pair(pending[0], pending[1], qaccT, rs_ps)
            pending = (p, et8)
        return pending

    def emit_epilogue(s, qaccT, raccs, rs_ps):
        row0 = s * MSUP
        rs_sb = small.tile([1, MSUP], F32, tag="rs_sb", name="rs_sb")
        if rs_on_pe:
            # denominators already accumulated on the PE
            nc.vector.tensor_copy(rs_sb, rs_ps[0:1, :])
        else:
            # softmax denominators: fold racc over partitions
            racc = raccs[0]
            if len(raccs) > 1:
                nc.vector.tensor_add(racc, racc, raccs[1])
            rst = psum_g1.tile([P, MSUP], F32, tag="ps", name="rst")
            nc.tensor.matmul(rst[0:1, :], ones_f, racc, start=True, stop=True)
            nc.vector.tensor_copy(rs_sb, rst[0:1, :])
        rcol = small.tile([P, MTS], F32, tag="rcol", name="rcol")
        for mt in range(MTS):
            rtp = psum_t.tile([P, P], F32, tag="pst", name="rtp")
            nc.tensor.transpose(
                rtp[:, 0:1], rs_sb[0:1, mt * P:(mt + 1) * P], ident_f[0:1, 0:1]
            )
            nc.vector.tensor_copy(rcol[:, mt:mt + 1], rtp[:, 0:1])
        rr = small.tile([P, MTS], F32, tag="rr", name="rr")
        nc.vector.reciprocal(rr, rcol)

        # Q.T -> natural layout, scaled by 1/rowsum. Split the accumulator
        # drain across DVE and ACT so the PSUM banks free up faster.
        qsb = io_pool.tile([P, DT, MSUP], F32, tag="qsb", name="qsb")
        for dd in range(DT):
            nc.vector.tensor_copy(qsb[:, dd, :], qaccT[:, dd, :])
        for mt in range(MTS):
            onat = io_pool.tile([P, d], F32, tag="onat", name="onat")
            for dd in range(DT):
                qtp = psum_t.tile([P, P], F32, tag="pst", name="qtp")
                nc.tensor.transpose(
                    qtp, qsb[:, dd, mt * P:(mt + 1) * P], ident_f
                )
                nc.vector.tensor_scalar_mul(
                    onat[:, dd * P:(dd + 1) * P], qtp, rr[:, mt:mt + 1]
                )
            nc.sync.dma_start(
                out_ap[row0 + mt * P:row0 + (mt + 1) * P, :], onat
            )

    # Software-pipelined super-tile loop: super s+1's normalize/xnT block is
    # emitted mid-way through super s's k loop so ACT/DVE precompute it while
    # the PE is saturated with matmuls, killing the boundary stall.
    xnT = emit_norm_xnT(0)
    for s in range(MS):
        qaccT = psum_q.tile([P, DT, MSUP], F32, name="qaccT")  # 4 banks
        rs_ps = (psum_rs.tile([P, MSUP], F32, name="rs_ps")
                 if rs_on_pe else None)
        if rs_on_pe:
            raccs = None
        else:
            n_racc = 2 if (RACC_SPLIT or (G2_FP8 and RACC_GPS_SPLIT)) else 1
            raccs = [
                racc_pool.tile([P, MSUP], F32, name=f"racc{i}", tag=f"racc{i}")
                for i in range(n_racc)
            ]
        if G2_FP8:
            pending = emit_kloop_segment_fp8(
                range(0, NPAIR // 2), xnT, qaccT, raccs, rs_ps, None)
            next_xnT = emit_norm_xnT(s + 1) if s + 1 < MS else None
            pending = emit_kloop_segment_fp8(
                range(NPAIR // 2, NPAIR), xnT, qaccT, raccs, rs_ps, pending)
            if pending is not None:
                emit_g2_pair(pending[0], pending[1], qaccT, rs_ps)
        else:
            pending = emit_kloop_segment(
                range(0, KT // 2), xnT, qaccT, raccs, None)
            next_xnT = emit_norm_xnT(s + 1) if s + 1 < MS else None
            pending = emit_kloop_segment(range(KT // 2, KT), xnT, qaccT, raccs,
                                         pending)
            if pending is not None:
                emit_g2(pending[0], pending[1], qaccT)
        emit_epilogue(s, qaccT, raccs, rs_ps)
        xnT = next_xnT


def _build_kernel_inner_t2(ctx, tc, out_ap, x_ap, cbt_ap, cb8_ap, n_local,
                           k, d):
    """Natural-output fp8 dataflow (no epilogue transposes).

    Host pre-casts the codebook: cbt_ap [P, DT, k] bf16 (transposed, GEMM1
    stationary) and cb8_ap [P, KT, d] fp8e4 (CB8_SCALE-scaled, GEMM2 moving
    operand). Per super-tile of 512 rows:
      GEMM1 (bf16): simsT chunk [k128, m512] = cb_t.T @ xnT, exp'd on ACT
        straight into fp8 pair tiles et8 [128, 2, 512].
      GEMM2 (fp8 DoubleRow, swapped): for each m-block mb, stationary =
        et8[:, :, mb128] and moving = cb8 pair [128, 2, 512] accumulate
        qacc[m128, d512] in natural layout. One extra DoubleRow ones-matmul
        per pair accumulates the softmax denominators rs [1, m512] on the PE.
      Epilogue: rs -> 4 tiny transposes -> rr [m,1]; osb = qacc * rr; DMA out.
    PSUM: 2 (g1) + 4 (qacc) + 1 (rs) + 1 (tiny transposes) = 8 banks.
    """
    nc = tc.nc
    P = 128
    KT = k // P          # 64
    DT = d // P          # 4
    MSUP = 512
    MTS = MSUP // P      # 4
    MS = n_local // MSUP  # 8
    NPAIR = KT // 2      # 32

    persist = ctx.enter_context(tc.tile_pool(name="persist", bufs=1))
    io_pool = ctx.enter_context(tc.tile_pool(name="io", bufs=2))
    expt_pool = ctx.enter_context(tc.tile_pool(name="expt", bufs=EXPT_BUFS))
    small = ctx.enter_context(tc.tile_pool(name="small", bufs=4))
    psum_t = ctx.enter_context(tc.tile_pool(name="psum_t", bufs=1,
                                            space="PSUM"))
    psum_g1 = ctx.enter_context(
        tc.tile_pool(name="psum_g1", bufs=PSUM_G1_BUFS, space="PSUM"))
    psum_q = ctx.enter_context(tc.tile_pool(name="psum_q", bufs=1,
                                            space="PSUM"))
    psum_rs = ctx.enter_context(tc.tile_pool(name="psum_rs", bufs=1,
                                             space="PSUM"))

    ident = persist.tile([P, P], BF16)
    make_identity(nc, ident)
    ident_f = persist.tile([P, P], F32)
    make_identity(nc, ident_f)
    ones8 = persist.tile([P, 2, 16], FP8)
    nc.vector.memset(ones8, CB8_SCALE)

    cb_t = persist.tile([P, DT, k], BF16)
    nc.sync.dma_start(cb_t, cbt_ap)
    cb8 = persist.tile([P, KT, d], FP8)
    nc.sync.dma_start(cb8, cb8_ap)

    inv_tau = float(1.0 / TAU)

    et8_const = None
    if ABLATE == "noact":
        et8_const = persist.tile([P, 2, MSUP], FP8, name="et8c")
        nc.vector.memset(et8_const, 1.0)
    ps_const = None
    if ABLATE == "nog1":
        pass

    def emit_norm_xnT_mt(s, mt, xnT):
        row0 = s * MSUP
        x_t = io_pool.tile([P, d], F32, name="x_t")
        nc.sync.dma_start(x_t, x_ap[row0 + mt * P:row0 + (mt + 1) * P, :])
        sq = io_pool.tile([P, d], F32, name="sq")
        ss = small.tile([P, 1], F32, name="ss")
        if NORM_ON_GPSIMD:
            nc.gpsimd.tensor_mul(sq, x_t, x_t)
            nc.vector.tensor_reduce(ss, sq, axis=mybir.AxisListType.X,
                                    op=ALU.add)
        else:
            nc.scalar.activation(out=sq, in_=x_t, func=AF.Square, accum_out=ss)
        nrm = small.tile([P, 1], F32, name="nrm")
        nc.scalar.sqrt(nrm, ss)
        rstd = small.tile([P, 1], F32, name="rstd")
        nc.vector.reciprocal(rstd, nrm)
        xn_b = io_pool.tile([P, d], BF16, name="xn_b")
        nc.vector.tensor_scalar_mul(xn_b, x_t, rstd)
        for dd in range(DT):
            if XNT_DMA_TRANSPOSE:
                nc.sync.dma_start_transpose(
                    xnT[:, dd, mt * P:(mt + 1) * P],
                    xn_b[:, dd * P:(dd + 1) * P])
            else:
                xps = psum_t.tile([P, P], BF16, tag="pst", name="xps")
                nc.tensor.transpose(xps, xn_b[:, dd * P:(dd + 1) * P], ident)
                nc.vector.tensor_copy(xnT[:, dd, mt * P:(mt + 1) * P], xps)

    def emit_norm_xnT(s):
        xnT = io_pool.tile([P, DT, MSUP], BF16, name="xnT")
        for mt in range(MTS):
            emit_norm_xnT_mt(s, mt, xnT)
        return xnT

    def emit_g2_pair(p, et8, qacc, rs_ps):
        # swapped DoubleRow GEMM2: stationary = exp m-block, moving = cb8
        # pair -> natural [m, d] output
        for mb in range(MTS):
            nc.tensor.matmul(
                qacc[:, mb, :],
                et8[:, :, mb * P:(mb + 1) * P],
                cb8[:, 2 * p:2 * p + 2, :],
                start=(p == 0),
                stop=(p == NPAIR - 1),
                perf_mode=mybir.MatmulPerfMode.DoubleRow,
            )
        # softmax denominators: rs[0, m] += 32 * colsum(et8)
        nc.tensor.matmul(
            rs_ps[0:1, :],
            ones8[:, :, 0:1],
            et8,
            start=(p == 0),
            stop=(p == NPAIR - 1),
            perf_mode=mybir.MatmulPerfMode.DoubleRow,
        )

    def emit_kloop_segment(pair_range, xnT, qacc, rs_ps, pending):
        # pending: list of up to G2_DELAY_PAIRS deferred (p, et8) entries.
        # Deferring GEMM2 keeps the et8 stationary tile written well before
        # its LDWEIGHTS issues, so the weight load never waits on ACT.
        for p in pair_range:
            if ABLATE == "noact":
                for j in (0, 1):
                    kk = 2 * p + j
                    ps = psum_g1.tile([P, MSUP], F32, name="ps")
                    for dd in range(DT):
                        nc.tensor.matmul(
                            ps, cb_t[:, dd, kk * P:(kk + 1) * P],
                            xnT[:, dd, :], start=(dd == 0),
                            stop=(dd == DT - 1))
                et8 = et8_const
            elif ABLATE == "nog1":
                et8 = expt_pool.tile([P, 2, MSUP], FP8, name="et8")
                for j in (0, 1):
                    nc.scalar.activation(
                        out=et8[:, j, :], in_=xnT[:, 0, :], func=AF.Exp,
                        scale=inv_tau)
            elif G2_INTERLEAVE and pending and len(pending) > G2_DELAY_PAIRS:
                # fine interleave: feed the PE alternating independent
                # accumulation chains (2x G1 : 1x G2') to hide PSUM
                # turnaround, instead of block-emitting 8 G1 then 5 G2'.
                pd_p, pd_et8 = pending.pop(0)
                g2q = [
                    lambda mb=mb: nc.tensor.matmul(
                        qacc[:, mb, :],
                        pd_et8[:, :, mb * P:(mb + 1) * P],
                        cb8[:, 2 * pd_p:2 * pd_p + 2, :],
                        start=(pd_p == 0), stop=(pd_p == NPAIR - 1),
                        perf_mode=mybir.MatmulPerfMode.DoubleRow)
                    for mb in range(MTS)
                ] + [
                    lambda: nc.tensor.matmul(
                        rs_ps[0:1, :], ones8[:, :, 0:1], pd_et8,
                        start=(pd_p == 0), stop=(pd_p == NPAIR - 1),
                        perf_mode=mybir.MatmulPerfMode.DoubleRow)
                ]
                et8 = expt_pool.tile([P, 2, MSUP], FP8, name="et8")
                gi = 0
                for j in (0, 1):
                    kk = 2 * p + j
                    ps = psum_g1.tile([P, MSUP], F32, name="ps")
                    for dd in range(DT):
                        nc.tensor.matmul(
                            ps,
                            cb_t[:, dd, kk * P:(kk + 1) * P],
                            xnT[:, dd, :],
                            start=(dd == 0),
                            stop=(dd == DT - 1),
                        )
                        if dd % 2 == 1 and gi < len(g2q):
                            g2q[gi]()
                            gi += 1
                    nc.scalar.activation(
                        out=et8[:, j, :], in_=ps, func=AF.Exp, scale=inv_tau)
                while gi < len(g2q):
                    g2q[gi]()
                    gi += 1
            else:
                et8 = expt_pool.tile([P, 2, MSUP], FP8, name="et8")
                for j in (0, 1):
                    kk = 2 * p + j
                    ps = psum_g1.tile([P, MSUP], F32, name="ps")
                    for dd in range(DT):
                        nc.tensor.matmul(
                            ps,
                            cb_t[:, dd, kk * P:(kk + 1) * P],
                            xnT[:, dd, :],
                            start=(dd == 0),
                            stop=(dd == DT - 1),
                        )
                    nc.scalar.activation(
                        out=et8[:, j, :], in_=ps, func=AF.Exp, scale=inv_tau)
            if ABLATE == "nog2":
                continue
            pending.append((p, et8))
            if len(pending) >= G2_DELAY_PAIRS + 1 and not (
                    G2_INTERLEAVE and ABLATE == ""):
                pd = pending.pop(0)
                emit_g2_pair(pd[0], pd[1], qacc, rs_ps)
        return pending

    def emit_epilogue(s, qacc, rs_ps):
        row0 = s * MSUP
        if ABLATE == "nog2":
            for mb in range(MTS):
                osb = io_pool.tile([P, d], F32, tag="osb", name="osb")
                nc.vector.memset(osb, 0.0)
                nc.sync.dma_start(
                    out_ap[row0 + mb * P:row0 + (mb + 1) * P, :], osb)
            return
        rs_sb = small.tile([1, MSUP], F32, tag="rs_sb", name="rs_sb")
        nc.vector.tensor_copy(rs_sb, rs_ps[0:1, :])
        rcol = small.tile([P, MTS], F32, tag="rcol", name="rcol")
        for mt in range(MTS):
            rtp = psum_t.tile([P, P], F32, tag="pst", name="rtp")
            nc.tensor.transpose(
                rtp[:, 0:1], rs_sb[0:1, mt * P:(mt + 1) * P], ident_f[0:1, 0:1]
            )
            nc.vector.tensor_copy(rcol[:, mt:mt + 1], rtp[:, 0:1])
        rr = small.tile([P, MTS], F32, tag="rr", name="rr")
        nc.vector.reciprocal(rr, rcol)
        for mb in range(MTS):
            osb = io_pool.tile([P, d], F32, tag="osb", name="osb")
            nc.vector.tensor_scalar_mul(osb, qacc[:, mb, :], rr[:, mb:mb + 1])
            nc.sync.dma_start(
                out_ap[row0 + mb * P:row0 + (mb + 1) * P, :], osb
            )

    xnT = emit_norm_xnT(0)
    for s in range(MS):
        qacc = psum_q.tile([P, MTS, d], F32, name="qacc")  # 4 banks
        rs_ps = psum_rs.tile([P, MSUP], F32, name="rs_ps")
        pending = emit_kloop_segment(range(0, NPAIR // 2), xnT, qacc, rs_ps,
                                     [])
        next_xnT = emit_norm_xnT(s + 1) if s + 1 < MS else None
        pending = emit_kloop_segment(range(NPAIR // 2, NPAIR), xnT, qacc,
                                     rs_ps, pending)
        for pd in pending:
            emit_g2_pair(pd[0], pd[1], qacc, rs_ps)
        emit_epilogue(s, qacc, rs_ps)
        xnT = next_xnT


def _build_kernel_inner(ctx, tc, out_ap, x_ap, cb_ap, n_local, k, d):
    nc = tc.nc
    P = 128
    KT = k // P          # 64  k-tiles (codebook rows per partition-chunk)
    DT = d // P          # 4   d-tiles
    NCH = k // 512       # 16  512-wide chunks of the sims row
    MT = n_local // P    # 32  row tiles per core

    persist = ctx.enter_context(tc.tile_pool(name="persist", bufs=1))
    stage = ctx.enter_context(tc.tile_pool(name="stage", bufs=3))
    io_pool = ctx.enter_context(tc.tile_pool(name="io", bufs=2))
    exp_pool = ctx.enter_context(tc.tile_pool(name="exp", bufs=2))
    st_pool = ctx.enter_context(tc.tile_pool(name="st", bufs=8))
    small = ctx.enter_context(tc.tile_pool(name="small", bufs=4))
    psum_t = ctx.enter_context(tc.tile_pool(name="psum_t", bufs=3, space="PSUM"))
    psum_g1 = ctx.enter_context(tc.tile_pool(name="psum_g1", bufs=2, space="PSUM"))
    psum_q = ctx.enter_context(tc.tile_pool(name="psum_q", bufs=2, space="PSUM"))

    ident = persist.tile([P, P], BF16)
    make_identity(nc, ident)

    # codebook, natural [k, d] layout, partition-chunked over k, bf16
    cb_n = persist.tile([P, KT, d], BF16)
    # codebook transposed to [d, k], partition-chunked over d, bf16
    cb_t = persist.tile([P, DT, k], BF16)

    for ko in range(KT):
        cst = stage.tile([P, d], F32)
        nc.sync.dma_start(cst, cb_ap[ko * P:(ko + 1) * P, :])
        nc.gpsimd.tensor_copy(cb_n[:, ko, :], cst)
        for dd in range(DT):
            if USE_DMA_TRANSPOSE:
                nc.sync.dma_start(
                    cb_t[:, dd, ko * P:(ko + 1) * P],
                    cb_n[:, ko, dd * P:(dd + 1) * P],
                    transpose=True,
                )
            else:
                tps = psum_t.tile([P, P], BF16, tag="pst")
                nc.tensor.transpose(tps, cb_n[:, ko, dd * P:(dd + 1) * P], ident)
                nc.vector.tensor_copy(cb_t[:, dd, ko * P:(ko + 1) * P], tps)

    inv_tau = float(1.0 / TAU)

    for m in range(MT):
        row0 = m * P
        # ---- load + normalize ----
        x_t = io_pool.tile([P, d], F32)
        nc.sync.dma_start(x_t, x_ap[row0:row0 + P, :])
        sq = io_pool.tile([P, d], F32)
        ss = small.tile([P, 1], F32)
        nc.scalar.activation(out=sq, in_=x_t, func=AF.Square, accum_out=ss)
        nrm = small.tile([P, 1], F32)
        nc.scalar.sqrt(nrm, ss)
        rstd = small.tile([P, 1], F32)
        nc.vector.reciprocal(rstd, nrm)
        xn_b = io_pool.tile([P, d], BF16)
        nc.vector.tensor_scalar_mul(xn_b, x_t, rstd)

        # ---- transpose xn -> lhsT for GEMM1 ----
        xnT = io_pool.tile([P, DT, P], BF16)
        for dd in range(DT):
            if USE_DMA_TRANSPOSE:
                nc.sync.dma_start(
                    xnT[:, dd, :], xn_b[:, dd * P:(dd + 1) * P], transpose=True
                )
            else:
                xps = psum_t.tile([P, P], BF16, tag="pst")
                nc.tensor.transpose(xps, xn_b[:, dd * P:(dd + 1) * P], ident)
                nc.scalar.copy(xnT[:, dd, :], xps)

        # ---- GEMM1 + exp ----
        exp_b = exp_pool.tile([P, k], BF16)
        parts = small.tile([P, NCH], F32)
        for n in range(NCH):
            ps = psum_g1.tile([P, 512], F32)
            for dd in range(DT):
                nc.tensor.matmul(
                    ps,
                    xnT[:, dd, :],
                    cb_t[:, dd, n * 512:(n + 1) * 512],
                    start=(dd == 0),
                    stop=(dd == DT - 1),
                )
            nc.scalar.activation(
                out=exp_b[:, n * 512:(n + 1) * 512],
                in_=ps,
                func=AF.Exp,
                scale=inv_tau,
                accum_out=parts[:, n:n + 1],
            )

        rs = small.tile([P, 1], F32)
        nc.vector.tensor_reduce(rs, parts, axis=mybir.AxisListType.X, op=ALU.add)
        rr = small.tile([P, 1], F32)
        nc.vector.reciprocal(rr, rs)

        # ---- GEMM2: q_unnorm = exp @ cb ----
        qacc = psum_q.tile([P, d], F32)
        for kk in range(KT):
            st = st_pool.tile([P, P], BF16)
            if USE_DMA_TRANSPOSE:
                nc.sync.dma_start(
                    st, exp_b[:, kk * P:(kk + 1) * P], transpose=True
                )
            else:
                pst = psum_t.tile([P, P], BF16, tag="pst")
                nc.tensor.transpose(pst, exp_b[:, kk * P:(kk + 1) * P], ident)
                nc.vector.tensor_copy(st, pst)
            nc.tensor.matmul(
                qacc, st, cb_n[:, kk, :], start=(kk == 0), stop=(kk == KT - 1)
            )

        # ---- epilogue: fold softmax denominator into output scale ----
        o_sb = io_pool.tile([P, d], F32)
        nc.vector.tensor_scalar_mul(o_sb, qacc, rr)
        nc.sync.dma_start(out_ap[row0:row0 + P, :], o_sb)


def build_bass(n_local, k=K_FULL, d=D_FULL, n_cores=N_CORES, reps=1):
    nc = bacc.Bacc(
        "TRN2",
        target_bir_lowering=False,
        debug=False,
        num_devices=n_cores,
    )
    x_ap = nc.dram_tensor("x", [n_local, d], F32, kind="ExternalInput").ap()
    if LAYOUT == "tr2" and n_local % 512 == 0:
        # host pre-casts the (constant) codebook into the two on-device
        # layouts, so the kernel skips the cast/transpose setup phase
        cbt_ap = nc.dram_tensor(
            "codebook_t", [128, d // 128, k], BF16, kind="ExternalInput").ap()
        cb8_ap = nc.dram_tensor(
            "codebook8", [128, k // 128, d], FP8, kind="ExternalInput").ap()
        cb_aps = (cbt_ap, cb8_ap)
    else:
        cb_ap = nc.dram_tensor(
            "codebook", [k, d], F32, kind="ExternalInput").ap()
        cb_aps = (cb_ap,)
    out_ap = nc.dram_tensor("out", [n_local, d], F32, kind="ExternalOutput").ap()
    with tile.TileContext(nc) as tc:
        _build_kernel(tc, out_ap, x_ap, cb_aps, n_local, k, d, reps=reps)
    nc.compile()
    return nc


_NC_CACHE = {}


def _get_nc(n_local, k, d, n_cores, reps=1):
    key = (n_local, k, d, n_cores, reps, USE_DMA_TRANSPOSE, LAYOUT, G2_FP8,
           RS_ON_PE, NORM_ON_GPSIMD, RACC_GPS_SPLIT, G2_DELAY_PAIRS,
           EXPT_BUFS, ABLATE, G2_INTERLEAVE, XNT_DMA_TRANSPOSE)
    if key not in _NC_CACHE:
        _NC_CACHE[key] = build_bass(n_local, k, d, n_cores, reps=reps)
    return _NC_CACHE[key]


def make_in_maps(x, codebook, n_cores=N_CORES):
    n, d = x.shape
    k = codebook.shape[0]
    n_local = n // n_cores
    cb = np.ascontiguousarray(codebook, dtype=np.float32)
    if LAYOUT == "tr2" and n_local % 512 == 0:
        import ml_dtypes

        P = 128
        cbt = np.ascontiguousarray(
            cb.T.reshape(d // P, P, k).transpose(1, 0, 2)
        ).astype(ml_dtypes.bfloat16)
        cb8 = np.ascontiguousarray(
            (cb * CB8_SCALE).reshape(k // P, P, d).transpose(1, 0, 2)
        ).astype(mybir.dt.np(FP8))
        shared = {"codebook_t": cbt, "codebook8": cb8}
    else:
        shared = {"codebook": cb}
    return [
        {
            "x": np.ascontiguousarray(x[i * n_local:(i + 1) * n_local],
                                      dtype=np.float32),
            **shared,
        }
        for i in range(n_cores)
    ]


def run_sharded(x, codebook, trace=False, reps=1):
    n, d = x.shape
    k = codebook.shape[0]
    assert n % N_CORES == 0
    n_local = n // N_CORES
    nc = _get_nc(n_local, k, d, N_CORES, reps=reps)
    in_maps = make_in_maps(x, codebook)
    res = run_bass_kernel_spmd(
        nc, in_maps, core_ids=list(range(N_CORES)), trace=trace
    )
    out = np.concatenate([r["out"] for r in res.results], axis=0)
    return out, res


def kernel(x, codebook):
    out, _ = run_sharded(x, codebook, trace=False)
    return out

